# revision 22
# baseline (speedup 1.0000x reference)
"""Trainium2 Bass kernel for the GNN message-passing module.

Per-sample pipeline (data-parallel: one batch element per NeuronCore):
  1. segment sums/counts via one-hot matmul on PE (x transposed on-chip),
  2. small "middle" stage: means, M=W@W^T, Mahalanobis adjacency folded
     into a (K, C_out) table: table2T = adj-weighted conv'd means,
  3. out = conv_w @ x + table2T[index] via PE matmuls (the gather is a
     one-hot matmul accumulated into the same PSUM as the 1x1 conv).

Math notes:
  adj[i,j] = exp(-(m_j-m_i)^T M (m_j-m_i)) with zero diagonal, M=W W^T.
  Using G = means @ M @ means^T, g = diag(G):
    adj[i,j] = exp(2G_ij - g_i - g_j) - delta_ij
  agg = adj @ means  =>  out += conv_w @ agg[index]
  table2T[k,:] = e^{-g_k} * (aggT_raw^T @ conv_w^T)[k,:] - (means @ conv_w^T)[k,:]
  where aggT_raw[:,i] = sum_j B[j,i] * (e^{-g_j} means[j,:]),
        B[i,j] = exp(2G_ij - g_i).
"""

import os
import sys

import numpy as np


def _ensure_path():
    try:
        import concourse  # noqa: F401
    except ImportError:
        for p in ("/opt/trn_rl_repo", os.path.expanduser("~/.axon_site/_ro/trn_rl_repo")):
            if os.path.isdir(p) and p not in sys.path:
                sys.path.insert(0, p)


_ensure_path()
# persistent jax/XLA executable cache: makes repeat compiles of the same
# kernel cheap across processes (first compile of a variant is ~minutes).
os.environ.setdefault("JAX_COMPILATION_CACHE_DIR", "/tmp/jax_neff_cache")
os.environ.setdefault("JAX_PERSISTENT_CACHE_MIN_COMPILE_TIME_SECS", "10")

import concourse.bass as bass  # noqa: E402
import concourse.tile as tile  # noqa: E402
from concourse import bacc  # noqa: E402
from concourse import mybir  # noqa: E402
from concourse.masks import make_identity  # noqa: E402

F32 = mybir.dt.float32
F32R = mybir.dt.float32r

# --- workaround: this walrus build rejects instructions carrying >2 sem
# waits ("Too many sync wait commands" in setupSyncWait). TileContext's exit
# drain accumulates one wait per outstanding processor (DMA queues etc.), so
# split them across NOPs emitted just before the drain. Semaphores are
# monotonic, so waiting earlier on the same conditions is equivalent.
_MAX_WAITS = 1
_drain_patched = False


def _patch_tile_drain():
    global _drain_patched
    if _drain_patched:
        return
    _drain_patched = True
    from concourse.vector_clock import ScopedClock

    orig = tile.TileContext._drain_and_barrier

    def patched(self, tick_clock, wait_clock):
        nc = self.nc
        probe = nc.sync.nop()
        wait_clock.add_sem_waits(
            probe.ins, ScopedClock({None: tick_clock.global_clock})
        )
        waits = list(probe.ins.sync_info.on_wait or [])
        chunks = [waits[i:i + _MAX_WAITS] for i in range(0, len(waits), _MAX_WAITS)]
        probe.ins.sync_info.on_wait = chunks[0] if chunks else []
        for chunk in chunks[1:]:
            nop = nc.sync.nop()
            nop.ins.sync_info = mybir.SyncInfo(on_wait=chunk, on_update=[])
        orig(self, tick_clock, wait_clock)
        _trim_redundant_waits(nc)

    tile.TileContext._drain_and_barrier = patched


def _trim_redundant_waits(nc):
    """Transitive wait reduction. Tile's add_semaphores is per-instruction
    minimal but not transitively minimal across processors: an instruction
    often carries waits already implied by (a) an earlier wait on the same
    engine, or (b) the closure of another wait it carries (the producer's own
    waits + in-order retirement on the producer's engine). This walrus build
    rejects instructions with >2 sync waits, so prune implied waits.

    Soundness assumptions: sem updates fire at instruction retirement;
    retirement is in-order per compute engine and per DMA queue sem (one sem
    per queue, FIFO); a kept wait on sem S>=v implies the v-reaching update's
    instruction retired, hence its dispatch-time holds and (non-DMA) all
    earlier same-engine updates.
    """
    import bisect

    for blk in nc.m.functions[0].blocks:
        insts = list(blk.instructions)
        n = len(insts)
        # sems that are ever decremented/reset are not monotonic; leave all
        # waits on them untouched and exclude them from closures (barrier
        # gather/release sems, end-of-kernel sem clears).
        nonmono = set()
        for ins in insts:
            si = ins.sync_info
            if si and si.on_update:
                for u in si.on_update:
                    if u.update_mode != "sem-inc":
                        nonmono.add(u.id)
            try:
                if ins.is_reset_sema:
                    lo = ins.reset_range_start
                    hi = ins.reset_range_stop
                    if lo is not None and hi is not None:
                        nonmono.update(range(lo, hi + 1))
            except Exception:
                pass
        upd = {}
        cum = {}
        own_cum_after = [None] * n
        eng_of = [str(i.engine) for i in insts]
        is_dma = [type(i).__name__ == "InstDMACopy" for i in insts]
        for idx, ins in enumerate(insts):
            si = ins.sync_info
            d = {}
            if si and si.on_update:
                for u in si.on_update:
                    if (u.update_mode != "sem-inc" or not u.update_value
                            or u.id in nonmono):
                        continue
                    c = cum.get(u.id, 0) + u.update_value
                    cum[u.id] = c
                    upd.setdefault(u.id, []).append((c, idx))
                    d[u.id] = c
            own_cum_after[idx] = d
        eng_cum_after = [None] * n
        run = {}
        for idx in range(n):
            e = eng_of[idx]
            m = dict(run.get(e, {}))
            if not is_dma[idx]:
                for s, c in own_cum_after[idx].items():
                    m[s] = c
            run[e] = m
            eng_cum_after[idx] = m

        def updater_idx(sem, v):
            lst = upd.get(sem)
            if not lst:
                return None
            pos = bisect.bisect_left(lst, (v, -1))
            if pos == len(lst):
                return None
            return lst[pos][1]

        holds_at = [None] * n
        last_eng = {}
        memo = {}

        def completion_holds(uidx):
            if uidx in memo:
                return memo[uidx]
            h = dict(holds_at[uidx] or {})
            src_cum = own_cum_after[uidx] if is_dma[uidx] else eng_cum_after[uidx]
            for s, c in src_cum.items():
                if h.get(s, 0) < c:
                    h[s] = c
            memo[uidx] = h
            return h

        n_dropped = 0
        for idx, ins in enumerate(insts):
            e = eng_of[idx]
            base = dict(holds_at[last_eng[e]]) if e in last_eng else {}
            si = ins.sync_info
            if si and si.on_wait:
                kept = []
                for w in si.on_wait:
                    if w.wait_mode != "sem-ge-imm" or w.id in nonmono:
                        kept.append(w)
                        continue
                    if base.get(w.id, 0) >= w.wait_value:
                        n_dropped += 1
                        continue
                    kept.append(w)
                    ui = updater_idx(w.id, w.wait_value)
                    if ui is not None and ui < idx:
                        for s, v in completion_holds(ui).items():
                            if base.get(s, 0) < v:
                                base[s] = v
                    if base.get(w.id, 0) < w.wait_value:
                        base[w.id] = w.wait_value
                if len(kept) != len(si.on_wait):
                    si.on_wait = kept
            holds_at[idx] = base
            last_eng[e] = idx
_compile_patched = False


def _patch_compile_bir():
    """This walrus build accepts at most ONE sync wait per instruction in
    several encodings (S3_LW matmuls, CTRL NoOp/Drain). Tile legitimately
    emits 2 waits on some instructions, so rewrite the serialized BIR just
    before walrus: keep one wait on the instruction and hoist the rest onto
    same-engine NoOps inserted immediately before it (same dispatch point,
    so semantics are unchanged)."""
    global _compile_patched
    if _compile_patched:
        return
    _compile_patched = True
    import orjson

    from concourse import bass2jax, bass_utils

    orig = bass_utils.compile_bir_kernel

    def _split_waits(bir_json: bytes) -> bytes:
        d = orjson.loads(bir_json)
        changed = False
        for fn in d.get("functions", []):
            for blk in fn.get("blocks", []):
                insts = blk.get("instructions", [])
                out = []
                for inst in insts:
                    si = inst.get("sync_info") or {}
                    ow = si.get("on_wait") or []
                    if len(ow) > 1:
                        changed = True
                        for k, w in enumerate(ow[:-1]):
                            out.append({
                                "debug": inst.get("debug", 0),
                                "engine": inst["engine"],
                                "ins": [],
                                "name": f"{inst['name']}-w{k}",
                                "opcode": "NoOp",
                                "outs": [],
                                "sync_info": {"on_update": [],
                                              "on_wait": [w]},
                            })
                        si["on_wait"] = [ow[-1]]
                    out.append(inst)
                blk["instructions"] = out
        return orjson.dumps(d) if changed else bir_json

    def wrapper(bir_json, tmpdir, neff_name="file.neff"):
        return orig(_split_waits(bir_json), tmpdir, neff_name=neff_name)

    bass_utils.compile_bir_kernel = wrapper
    bass2jax.compile_bir_kernel = wrapper


AF = mybir.ActivationFunctionType
ALU = mybir.AluOpType

B, C, K, H, W_DIM = 8, 256, 64, 128, 128
HW = H * W_DIM  # 16384 pixels per sample
N_CORES = 8

# config knobs (perf iteration; env-overridable for experiments)
# bf16x3: x and conv_w arrive host-split into bf16 hi+lo planes; every hot
# matmul runs bf16 (1 cyc/row, FWL) with fp32 PSUM accumulation. Dropped
# lo*lo term => ~1.5e-5 relative error (fp32-envelope class).
USE_BF16X3 = os.environ.get("KERNEL_BF16X3", "1") == "1"
# lo-plane refinement: tolerance is 2e-2 and bf16-hi-only lands at 2.7e-3,
# so the lo planes (which triple pass-2 PE work) are off by default.
USE_LO = os.environ.get("KERNEL_LO", "0") == "1"
# XBAR DMA transpose for pass-1 x tiles (instead of PE transposes + PSUM
# round-trip): matmul self-load serializes with streaming, so keeping the
# PE free of transposes is worth ~27us.
USE_XBAR = os.environ.get("KERNEL_XBAR", "1") == "1"
USE_F32R_MM = os.environ.get("KERNEL_F32R_MM", "0") == "1"
USE_F32R_TR = os.environ.get("KERNEL_F32R_TR", "0") == "1"
PX_TILE = 2048        # pass-1 x DMA tile (pixels)
# pass-2 pixel tile: 512 is the max single-matmul PSUM span (one 2KB bank
# of fp32) — wider streams fail the walrus ISA check.
P2_TILE = int(os.environ.get("KERNEL_P2", "512"))


def _mm(ap):
    return ap.bitcast(F32R) if USE_F32R_MM else ap


def _tr(ap):
    return ap.bitcast(F32R) if USE_F32R_TR else ap


def build_nc():
    _patch_tile_drain()
    _patch_compile_bir()
    # Bacc (not raw Bass): its compile() pass auto-inserts the GPSIMD
    # library reloads that ap_gather needs, in correct program order.
    nc = bacc.Bacc("TRN2", target_bir_lowering=False, debug=False)
    BF16 = mybir.dt.bfloat16
    idx_d = nc.dram_tensor("idxf", (HW,), F32, kind="ExternalInput")
    wt_d = nc.dram_tensor("wt", (C, C), F32, kind="ExternalInput")      # W^T
    out_d = nc.dram_tensor("out", (C, HW), F32, kind="ExternalOutput")
    if USE_BF16X3:
        ins = dict(
            xh=nc.dram_tensor("xh", (C, HW), BF16, kind="ExternalInput").ap(),
            cwth=nc.dram_tensor("cwth", (C, C), BF16, kind="ExternalInput").ap(),
            cwt=nc.dram_tensor("cwt", (C, C), F32, kind="ExternalInput").ap(),
            ohT=nc.dram_tensor("ohT", (K, HW), BF16, kind="ExternalInput").ap(),
            recip=nc.dram_tensor("recip", (K, 1), F32, kind="ExternalInput").ap(),
        )
        if USE_LO:
            ins["xl"] = nc.dram_tensor("xl", (C, HW), BF16,
                                       kind="ExternalInput").ap()
            ins["cwtl"] = nc.dram_tensor("cwtl", (C, C), BF16,
                                         kind="ExternalInput").ap()
    else:
        ins = dict(
            x=nc.dram_tensor("x", (C, HW), F32, kind="ExternalInput").ap(),
            cwt=nc.dram_tensor("cwt", (C, C), F32, kind="ExternalInput").ap(),
            idxw=nc.dram_tensor("idxw", (128, HW // 16), mybir.dt.int16,
                                kind="ExternalInput").ap(),
        )

    with tile.TileContext(nc) as tc:
        _body(tc, ins, idx_d.ap(), wt_d.ap(), out_d.ap())
    nc.compile()
    return nc


def _body(tc, ins, idx_v, wt_v, out_v):
    nc = tc.nc
    BF16 = mybir.dt.bfloat16
    n_px_tiles = HW // PX_TILE              # 8
    chunks_per_px_tile = PX_TILE // 128     # 16
    n_p2_tiles = HW // P2_TILE              # 32
    cwt_v = ins["cwt"]

    with (
        tc.tile_pool(name="consts", bufs=1) as consts,
        tc.tile_pool(name="xres",
                     bufs=(2 if USE_LO else 1) * n_px_tiles) as xres,
        tc.tile_pool(name="mid_sb", bufs=1) as mid_sb,
    ):
        # ---- constants / parameter loads ----
        ident = consts.tile([128, 128], F32, tag="ident")
        make_identity(nc, ident[:])
        identb = consts.tile([128, 128], BF16, tag="identb")
        make_identity(nc, identb[:])

        iota_row = consts.tile([128, K], F32, tag="iota_row")  # [p,k] = k
        iota_row_i = consts.tile([128, K], mybir.dt.int32, tag="iota_row_i")
        nc.gpsimd.iota(iota_row_i[:], pattern=[[1, K]], base=0, channel_multiplier=0)
        nc.vector.tensor_copy(iota_row[:], iota_row_i[:])

        negI = consts.tile([K, K], F32, tag="negI")            # -identity(64)
        nc.gpsimd.memset(negI[:], 0.0)
        nc.gpsimd.affine_select(
            out=negI[:], in_=negI[:], compare_op=ALU.not_equal,
            fill=-1.0, base=0, pattern=[[-1, K]], channel_multiplier=1,
        )

        wt_sb = consts.tile([128, 2, C], F32, tag="wt_sb")     # [e, j, c] = W^T[j*128+e, c]
        nc.sync.dma_start(out=wt_sb[:], in_=wt_v.rearrange("(j p) c -> p j c", p=128))
        cwt_sb = consts.tile([128, 2, C], F32, tag="cwt_sb")   # [ci, j, co]
        nc.sync.dma_start(out=cwt_sb[:], in_=cwt_v.rearrange("(j p) c -> p j c", p=128))

        idx_pm = consts.tile([128, 128], F32, tag="idx_pm")    # [p,f] = idx[p*128+f]
        nc.sync.dma_start(out=idx_pm[:], in_=idx_v.rearrange("(p f) -> p f", p=128))

        if USE_BF16X3:
            cwth_sb = consts.tile([128, 2, C], BF16, tag="cwth_sb")
            nc.sync.dma_start(
                out=cwth_sb[:],
                in_=ins["cwth"].rearrange("(j p) c -> p j c", p=128))
            if USE_LO:
                cwtl_sb = consts.tile([128, 2, C], BF16, tag="cwtl_sb")
                nc.sync.dma_start(
                    out=cwtl_sb[:],
                    in_=ins["cwtl"].rearrange("(j p) c -> p j c", p=128))
            recip_sb = consts.tile([K, 1], F32, tag="recip_sb")
            nc.sync.dma_start(out=recip_sb[:], in_=ins["recip"][:, :])
            # pass-2 one-hot (k on partitions) comes precomputed from host:
            # same 2MB of DMA as the old index broadcast, but no on-chip
            # compare work.
            ohT_sb = consts.tile([K, HW], BF16, tag="ohT_sb")
            nc.sync.dma_start(out=ohT_sb[:], in_=ins["ohT"][:, :])
        else:
            ones64 = consts.tile([K, 128], F32, tag="ones64")
            nc.vector.memset(ones64[:], 1.0)
            idxw_sb = consts.tile([128, HW // 16], mybir.dt.int16, tag="idxw_sb")
            nc.sync.dma_start(out=idxw_sb[:], in_=ins["idxw"][:, :])

        M_sb = mid_sb.tile([128, 2, C], F32, tag="M_sb")       # M = W @ W^T (symmetric)
        idxT = mid_sb.tile([128, 128], F32, tag="idxT")        # [q,i] = idx[i*128+q]
        means = mid_sb.tile([K, C], F32, tag="means")
        meansT = mid_sb.tile([128, 2, K], F32, tag="meansT")
        Q_sb = mid_sb.tile([128, 2, K], F32, tag="Q_sb")
        aggT_sb = mid_sb.tile([128, 2, K], F32, tag="aggT_sb")
        B_sb = mid_sb.tile([K, K], F32, tag="B_sb")
        tmp64 = mid_sb.tile([K, K], F32, tag="tmp64")
        if not USE_BF16X3:
            eq0 = mid_sb.tile([K, 1], F32, tag="eq0")
            den = mid_sb.tile([K, 1], F32, tag="den")
            recip = mid_sb.tile([K, 1], F32, tag="recip")
        neg_g = mid_sb.tile([K, 1], F32, tag="neg_g")
        e_col = mid_sb.tile([K, 1], F32, tag="e_col")
        if USE_BF16X3:
            tableM = mid_sb.tile([K, C], F32, tag="tableM")
            table2T = mid_sb.tile([K, C], F32, tag="table2T")
            tabh = mid_sb.tile([K, C], BF16, tag="tabh")
            if USE_LO:
                tabl = mid_sb.tile([K, C], BF16, tag="tabl")
                tabr = mid_sb.tile([K, C], F32, tag="tabr")
        else:
            table2 = mid_sb.tile([128, 2, K], F32, tag="table2")   # [c_out, k]
            e_bc = mid_sb.tile([128, K], F32, tag="e_bc")
            tmp128 = mid_sb.tile([128, K], F32, tag="tmp128")
            diagE = mid_sb.tile([K, K], F32, tag="diagE")

        with (
            tc.tile_pool(name="psum_sums", bufs=1, space="PSUM") as pp_sums,
            tc.tile_pool(name="psum_mid", bufs=2, space="PSUM") as pp_mid,
        ):
            # bf16 path: counts/recip come precomputed from host, no ones col
            psum_sums = pp_sums.tile([K, C if USE_BF16X3 else C + 1], F32,
                                     tag="psum_sums")

            # Warm-up: make PE observe the POOL-produced identities before
            # the hot loop so pass-1 transposes don't each carry a POOL wait.
            warm = pp_mid.tile([128, 128], F32, tag="pm")
            nc.tensor.transpose(warm[:], ident[:], ident[:])
            warm2 = pp_mid.tile([128, 128], BF16, tag="pmb")
            nc.tensor.transpose(warm2[:], identb[:], identb[:])

            # M = W @ W^T: contract e; lhsT/rhs both W^T (e on partitions).
            for h in range(2):
                pm = pp_mid.tile([128, C], F32, tag="pm")
                for j in range(2):
                    nc.tensor.matmul(
                        pm[:], wt_sb[:, j, h * 128:(h + 1) * 128],
                        wt_sb[:, j, :], start=(j == 0), stop=(j == 1),
                    )
                nc.scalar.copy(M_sb[:, h, :], pm[:])

            # idxT: transpose idx_pm so column i = indices of pixel chunk i
            pi = pp_mid.tile([128, 128], F32, tag="pm")
            nc.tensor.transpose(pi[:], idx_pm[:], ident[:])
            nc.scalar.copy(idxT[:], pi[:])

            # ---- pass 1: segment sums over all pixels ----
            with (
                tc.tile_pool(name="psum_p1", bufs=2, space="PSUM") as pp1,
                tc.tile_pool(name="xt_pool", bufs=4) as xt_pool,
                tc.tile_pool(name="oh_pool", bufs=6) as oh_pool,
            ):
                first = True
                x_tiles = []
                if USE_BF16X3:
                    streams = [("xh", None), ("xl", None)]
                for t in range(n_px_tiles):
                    if USE_BF16X3:
                        xt_h = xres.tile([128, 2, PX_TILE], BF16, tag="xres")
                        xt_l = (xres.tile([128, 2, PX_TILE], BF16, tag="xres")
                                if USE_LO else None)
                        x_tiles.append((xt_h, xt_l))
                        for j in range(2):
                            nc.sync.dma_start(
                                out=xt_h[:, j, :],
                                in_=ins["xh"][j * 128:(j + 1) * 128,
                                              t * PX_TILE:(t + 1) * PX_TILE])
                            if USE_LO:
                                nc.sync.dma_start(
                                    out=xt_l[:, j, :],
                                    in_=ins["xl"][j * 128:(j + 1) * 128,
                                                  t * PX_TILE:(t + 1) * PX_TILE])
                    else:
                        xt_full = xres.tile([128, 2, PX_TILE], F32, tag="xres")
                        x_tiles.append(xt_full)
                        for j in range(2):
                            nc.sync.dma_start(
                                out=xt_full[:, j, :],
                                in_=ins["x"][j * 128:(j + 1) * 128,
                                             t * PX_TILE:(t + 1) * PX_TILE])
                    for quad in range(chunks_per_px_tile // 4):
                        if USE_BF16X3 and USE_XBAR:
                            # XBAR DMA transpose straight into SBUF: no PE
                            # work, no PSUM round-trip, no HBM traffic.
                            xTh = xt_pool.tile([128, 4, C], BF16, tag="xTh")
                            for c4 in range(4):
                                cc = quad * 4 + c4
                                for j in range(2):
                                    nc.sync.dma_start_transpose(
                                        out=xTh[:, c4, j * 128:(j + 1) * 128],
                                        in_=xt_h[:, j, cc * 128:(cc + 1) * 128])
                            for c4 in range(4):
                                cc = quad * 4 + c4
                                gchunk = t * chunks_per_px_tile + cc
                                oh = oh_pool.tile([128, K], BF16, tag="oh")
                                nc.vector.tensor_scalar(
                                    out=oh[:], in0=iota_row[:],
                                    scalar1=idxT[:, gchunk:gchunk + 1],
                                    scalar2=None, op0=ALU.is_equal)
                                nc.tensor.matmul(
                                    psum_sums[:], oh[:], xTh[:, c4, :],
                                    start=first, stop=(gchunk == HW // 128 - 1))
                                first = False
                        elif USE_BF16X3:
                            # one (128,1024) PSUM per stream: 4 chunks each
                            pxh = pp1.tile([128, 1024], BF16, tag="pxt")
                            pxl = (pp1.tile([128, 1024], BF16, tag="pxt")
                                   if USE_LO else None)
                            for c4 in range(4):
                                cc = quad * 4 + c4
                                for j in range(2):
                                    o = c4 * 256 + j * 128
                                    nc.tensor.transpose(
                                        pxh[:, o:o + 128],
                                        xt_h[:, j, cc * 128:(cc + 1) * 128],
                                        identb[:])
                                    if USE_LO:
                                        nc.tensor.transpose(
                                            pxl[:, o:o + 128],
                                            xt_l[:, j, cc * 128:(cc + 1) * 128],
                                            identb[:])
                            xTh = xt_pool.tile([128, 4, C], BF16, tag="xTh")
                            xTl = (xt_pool.tile([128, 4, C], BF16, tag="xTl")
                                   if USE_LO else None)
                            if quad % 2 == 0:
                                nc.scalar.copy(
                                    xTh[:],
                                    pxh[:].rearrange("p (a b) -> p a b", a=4))
                                if USE_LO:
                                    nc.vector.tensor_copy(
                                        xTl[:],
                                        pxl[:].rearrange("p (a b) -> p a b", a=4))
                            else:
                                nc.vector.tensor_copy(
                                    xTh[:],
                                    pxh[:].rearrange("p (a b) -> p a b", a=4))
                                if USE_LO:
                                    nc.scalar.copy(
                                        xTl[:],
                                        pxl[:].rearrange("p (a b) -> p a b", a=4))
                            for c4 in range(4):
                                cc = quad * 4 + c4
                                gchunk = t * chunks_per_px_tile + cc
                                oh = oh_pool.tile([128, K], BF16, tag="oh")
                                nc.vector.tensor_scalar(
                                    out=oh[:], in0=iota_row[:],
                                    scalar1=idxT[:, gchunk:gchunk + 1],
                                    scalar2=None, op0=ALU.is_equal)
                                last = gchunk == HW // 128 - 1
                                nc.tensor.matmul(
                                    psum_sums[:], oh[:], xTh[:, c4, :],
                                    start=first, stop=(last and not USE_LO))
                                if USE_LO:
                                    nc.tensor.matmul(
                                        psum_sums[:], oh[:], xTl[:, c4, :],
                                        start=False, stop=last)
                                first = False
                        else:
                            pxt = pp1.tile([128, 1024], F32, tag="pxt")
                            for c4 in range(4):
                                cc = quad * 4 + c4
                                for j in range(2):
                                    nc.tensor.transpose(
                                        pxt[:, c4 * 256 + j * 128:
                                            c4 * 256 + (j + 1) * 128],
                                        xt_full[:, j, cc * 128:(cc + 1) * 128],
                                        ident[:])
                            xT = xt_pool.tile([128, 4, C + 1], F32, tag="xT")
                            if quad % 2 == 0:
                                nc.scalar.copy(
                                    xT[:, :, 0:C],
                                    pxt[:].rearrange("p (a b) -> p a b", a=4))
                            else:
                                nc.vector.tensor_copy(
                                    xT[:, :, 0:C],
                                    pxt[:].rearrange("p (a b) -> p a b", a=4))
                            nc.gpsimd.memset(xT[:, :, C:C + 1], 1.0)
                            for c4 in range(4):
                                cc = quad * 4 + c4
                                gchunk = t * chunks_per_px_tile + cc
                                oh = oh_pool.tile([128, K], F32, tag="oh")
                                nc.vector.tensor_scalar(
                                    out=oh[:], in0=iota_row[:],
                                    scalar1=idxT[:, gchunk:gchunk + 1],
                                    scalar2=None, op0=ALU.is_equal)
                                nc.tensor.matmul(
                                    psum_sums[:], oh[:], xT[:, c4, :],
                                    start=first,
                                    stop=(gchunk == HW // 128 - 1))
                                first = False

            # ---- middle: means -> adjacency -> table ----
            if USE_BF16X3:
                nc.vector.tensor_scalar(
                    out=means[:], in0=psum_sums[:, 0:C], scalar1=recip_sb[:],
                    scalar2=None, op0=ALU.mult,
                )
            else:
                nc.vector.tensor_scalar(
                    out=eq0[:], in0=psum_sums[:, C:C + 1], scalar1=0.0,
                    scalar2=None, op0=ALU.is_equal,
                )
                nc.vector.tensor_add(den[:], psum_sums[:, C:C + 1], eq0[:])
                nc.vector.reciprocal(recip[:], den[:])
                nc.vector.tensor_scalar(
                    out=means[:], in0=psum_sums[:, 0:C], scalar1=recip[:],
                    scalar2=None, op0=ALU.mult,
                )

            # meansT (c on partitions)
            for h in range(2):
                pm = pp_mid.tile([128, K], F32, tag="pm")
                nc.tensor.transpose(
                    pm[:], means[:, h * 128:(h + 1) * 128], ident[0:K, 0:K],
                )
                nc.scalar.copy(meansT[:, h, :], pm[:])

            # Q = M @ means^T  (use symmetry of M for lhsT slicing)
            for h in range(2):
                pq = pp_mid.tile([128, K], F32, tag="pm")
                for dj in range(2):
                    nc.tensor.matmul(
                        pq[:], M_sb[:, dj, h * 128:(h + 1) * 128],
                        meansT[:, dj, :], start=(dj == 0), stop=(dj == 1),
                    )
                nc.scalar.copy(Q_sb[:, h, :], pq[:])

            # G = means @ Q  (64x64, symmetric)
            pg = pp_mid.tile([K, K], F32, tag="pm")
            for h in range(2):
                nc.tensor.matmul(
                    pg[:], meansT[:, h, :], Q_sb[:, h, :],
                    start=(h == 0), stop=(h == 1),
                )

            # -g = rowsum(G * (-I));  e_col = exp(-g);  B = exp(2G - g_i)
            nc.vector.scalar_tensor_tensor(
                out=tmp64[:], in0=pg[:], scalar=1.0, in1=negI[:],
                op0=ALU.mult, op1=ALU.mult, accum_out=neg_g[:],
            )
            nc.scalar.activation(e_col[:], neg_g[:], AF.Exp)
            nc.scalar.activation(B_sb[:], pg[:], AF.Exp, bias=neg_g[:], scale=2.0)

            # aggT_raw[c,i] = sum_j B[j,i] means[j,c]
            # (B[j,i] = exp(2G_ij - g_j) already carries e^{-g_j})
            if USE_BF16X3:
                for h in range(2):
                    pa = pp_mid.tile([128, K], F32, tag="pm")
                    nc.tensor.matmul(
                        pa[:], means[:, h * 128:(h + 1) * 128], B_sb[:],
                        start=True, stop=True,
                    )
                    nc.scalar.copy(aggT_sb[:, h, :], pa[:])
                # table2T[k, c_out] = e^{-g_k}*(aggT_raw^T@cwt)[k,:] - means@cwt
                pt2 = pp_mid.tile([K, C], F32, tag="pm")
                ptm = pp_mid.tile([K, C], F32, tag="pm")
                for j in range(2):
                    nc.tensor.matmul(
                        pt2[:], aggT_sb[:, j, :], cwt_sb[:, j, :],
                        start=(j == 0), stop=(j == 1),
                    )
                for j in range(2):
                    nc.tensor.matmul(
                        ptm[:], meansT[:, j, :], cwt_sb[:, j, :],
                        start=(j == 0), stop=(j == 1),
                    )
                nc.scalar.copy(tableM[:], ptm[:])
                nc.vector.scalar_tensor_tensor(
                    out=table2T[:], in0=pt2[:], scalar=e_col[:], in1=tableM[:],
                    op0=ALU.mult, op1=ALU.subtract,
                )
                # split the table into bf16 hi(+lo) for the bf16 gather matmul
                nc.vector.tensor_copy(tabh[:], table2T[:])
                if USE_LO:
                    nc.vector.tensor_sub(tabr[:], table2T[:], tabh[:])
                    nc.vector.tensor_copy(tabl[:], tabr[:])
            else:
                # e^{-g} broadcast along free k: ones^T @ (I*e)
                nc.vector.tensor_scalar(
                    out=diagE[:], in0=ident[0:K, 0:K], scalar1=e_col[:],
                    scalar2=None, op0=ALU.mult,
                )
                pb = pp_mid.tile([128, K], F32, tag="pm")
                nc.tensor.matmul(pb[:], ones64[:], diagE[:],
                                 start=True, stop=True)
                nc.scalar.copy(e_bc[:], pb[:])
                for h in range(2):
                    pa = pp_mid.tile([128, K], F32, tag="pm")
                    nc.tensor.matmul(
                        pa[:], means[:, h * 128:(h + 1) * 128], B_sb[:],
                        start=True, stop=True,
                    )
                    nc.vector.tensor_mul(tmp128[:], pa[:], e_bc[:])
                    nc.vector.tensor_sub(aggT_sb[:, h, :], tmp128[:],
                                         meansT[:, h, :])
                for h in range(2):
                    pt = pp_mid.tile([128, K], F32, tag="pm")
                    for j in range(2):
                        nc.tensor.matmul(
                            pt[:], cwt_sb[:, j, h * 128:(h + 1) * 128],
                            aggT_sb[:, j, :], start=(j == 0), stop=(j == 1),
                        )
                    nc.scalar.copy(table2[:, h, :], pt[:])

        # ---- pass 2: out = conv_w @ x + table[index] ----
        out_r = out_v.rearrange("(h p) w -> p h w", p=128)
        s16 = P2_TILE // 16
        with (
            tc.tile_pool(name="psum_p2", bufs=(2 if P2_TILE >= 1024 else 3),
                         space="PSUM") as pp2,
            tc.tile_pool(name="p2_sb", bufs=4) as p2_sb,
            tc.tile_pool(name="p2_g", bufs=4) as p2_g,
        ):
            for t2 in range(n_p2_tiles):
                pt_ = (t2 * P2_TILE) // PX_TILE
                off = (t2 * P2_TILE) % PX_TILE

                po = pp2.tile([128, 2 * P2_TILE], F32, tag="po")
                ot = p2_sb.tile([128, 2, P2_TILE], F32, tag="ot")
                if USE_BF16X3:
                    xt_h, xt_l = x_tiles[pt_]
                    oh2 = ohT_sb[:, t2 * P2_TILE:(t2 + 1) * P2_TILE]
                    for h in range(2):
                        sl = slice(h * P2_TILE, (h + 1) * P2_TILE)
                        hs = slice(h * 128, (h + 1) * 128)
                        for j in range(2):
                            nc.tensor.matmul(
                                po[:, sl], cwth_sb[:, j, hs],
                                xt_h[:, j, off:off + P2_TILE],
                                start=(j == 0), stop=False)
                        if USE_LO:
                            for j in range(2):
                                nc.tensor.matmul(
                                    po[:, sl], cwth_sb[:, j, hs],
                                    xt_l[:, j, off:off + P2_TILE],
                                    start=False, stop=False)
                            for j in range(2):
                                nc.tensor.matmul(
                                    po[:, sl], cwtl_sb[:, j, hs],
                                    xt_h[:, j, off:off + P2_TILE],
                                    start=False, stop=False)
                        nc.tensor.matmul(
                            po[:, sl], tabh[:, hs], oh2,
                            start=False, stop=not USE_LO)
                        if USE_LO:
                            nc.tensor.matmul(
                                po[:, sl], tabl[:, hs], oh2,
                                start=False, stop=True)
                    if t2 % 2 == 0:
                        nc.scalar.copy(
                            ot[:], po[:].rearrange("p (a b) -> p a b", a=2))
                    else:
                        nc.vector.tensor_copy(
                            ot[:], po[:].rearrange("p (a b) -> p a b", a=2))
                else:
                    x_tile = x_tiles[pt_]
                    for h in range(2):
                        sl = slice(h * P2_TILE, (h + 1) * P2_TILE)
                        tab_g = p2_g.tile([128, P2_TILE], F32, tag="tabg")
                        nc.gpsimd.ap_gather(
                            out_ap=tab_g[:], in_ap=table2[:, h, :],
                            idxs_ap=idxw_sb[:, t2 * s16:(t2 + 1) * s16],
                            channels=128, num_elems=K, d=1, num_idxs=P2_TILE,
                        )
                        for j in range(2):
                            nc.tensor.matmul(
                                po[:, sl], cwt_sb[:, j, h * 128:(h + 1) * 128],
                                x_tile[:, j, off:off + P2_TILE],
                                start=(j == 0), stop=(j == 1),
                            )
                        nc.vector.tensor_add(ot[:, h, :], po[:, sl], tab_g[:])
                nc.sync.dma_start(
                    out=out_r[:, :, t2 * P2_TILE:(t2 + 1) * P2_TILE], in_=ot[:],
                )




def _ensure_ntff_hook():
    """Register the axon NTFF profiling hook if the image's antenv lacks it."""
    try:
        from antenv.axon_hooks import get_axon_ntff_profile_hook  # noqa: F401
        return
    except ImportError:
        pass
    import types

    import antenv

    mod = types.ModuleType("antenv.axon_hooks")
    _hook = [None]
    mod.set_axon_ntff_profile_hook = lambda h: _hook.__setitem__(0, h)
    mod.get_axon_ntff_profile_hook = lambda: _hook[0]
    sys.modules["antenv.axon_hooks"] = mod
    antenv.axon_hooks = mod
    try:
        from trn_agent_boot.trn_boot import _ntff_profile_via_ctypes

        so = "/opt/axon/libaxon_pjrt.so"
        if os.path.exists(so):
            mod.set_axon_ntff_profile_hook(_ntff_profile_via_ctypes(so))
    except Exception:
        pass


_NC_CACHE = None
LAST_RESULT = None


def _get_nc():
    global _NC_CACHE
    if _NC_CACHE is None:
        _NC_CACHE = build_nc()
    return _NC_CACHE


def kernel(x, index, W, conv_w):
    """Full inputs in, full output out. Shards batch across 8 NeuronCores."""
    global LAST_RESULT
    from concourse.bass_utils import run_bass_kernel_spmd

    import ml_dtypes

    x = np.asarray(x, dtype=np.float32).reshape(B, C, HW)
    idx_i = np.asarray(index).reshape(B, HW)
    idxf = idx_i.astype(np.float32)
    wt = np.ascontiguousarray(np.asarray(W, dtype=np.float32).T)
    cwt = np.ascontiguousarray(
        np.asarray(conv_w, dtype=np.float32).reshape(C, C).T
    )

    nc = _get_nc()
    if USE_BF16X3:
        # split x and conv_w^T into bf16 hi(+lo) planes on the host: the
        # device computes hi@hi (+ hi@lo + lo@hi when KERNEL_LO=1) in bf16
        # with fp32 PSUM accumulation.
        xh = x.astype(ml_dtypes.bfloat16)
        cwth = cwt.astype(ml_dtypes.bfloat16)
        # per-sample segment-count reciprocals (used to turn sums into means)
        counts = np.stack([np.bincount(idx_i[b], minlength=K) for b in range(B)])
        recip = (1.0 / np.maximum(counts, 1)).astype(np.float32).reshape(B, K, 1)
        # pass-2 one-hot, k on partitions (bf16 0/1 exact)
        ohT = (idx_i[:, None, :] == np.arange(K)[None, :, None]).astype(
            ml_dtypes.bfloat16)
        in_maps = [
            {"xh": np.ascontiguousarray(xh[b]),
             "idxf": np.ascontiguousarray(idxf[b]),
             "ohT": np.ascontiguousarray(ohT[b]),
             "recip": np.ascontiguousarray(recip[b]),
             "wt": wt, "cwt": cwt, "cwth": cwth}
            for b in range(B)
        ]
        if USE_LO:
            xl = (x - xh.astype(np.float32)).astype(ml_dtypes.bfloat16)
            cwtl = (cwt - cwth.astype(np.float32)).astype(ml_dtypes.bfloat16)
            for b in range(B):
                in_maps[b]["xl"] = np.ascontiguousarray(xl[b])
                in_maps[b]["cwtl"] = cwtl
    else:
        # ap_gather wrapped layout: [p, s] = idx[s*16 + p%16], tiled over
        # the 8 q7 cores' 16-partition groups
        idxw = np.ascontiguousarray(
            np.tile(idx_i.reshape(B, HW // 16, 16).transpose(0, 2, 1),
                    (1, 8, 1)).astype(np.int16)
        )
        in_maps = [
            {"x": np.ascontiguousarray(x[b]),
             "idxf": np.ascontiguousarray(idxf[b]),
             "idxw": idxw[b], "wt": wt, "cwt": cwt}
            for b in range(B)
        ]
    trace = bool(int(os.environ.get("KERNEL_TRACE", "0")))
    if trace:
        try:
            _ensure_ntff_hook()
            res = run_bass_kernel_spmd(
                nc, in_maps, core_ids=list(range(N_CORES)), trace=True,
            )
        except Exception as e:  # profiling must never break the answer path
            print(f"kernel: trace run failed ({e!r}); rerunning untraced")
            res = run_bass_kernel_spmd(
                nc, in_maps, core_ids=list(range(N_CORES)), trace=False,
            )
    else:
        res = run_bass_kernel_spmd(
            nc, in_maps, core_ids=list(range(N_CORES)), trace=False,
        )
    LAST_RESULT = res
    out = np.stack([res.results[b]["out"].reshape(C, H, W_DIM) for b in range(B)])
    return out

